# revision 1
# baseline (speedup 1.0000x reference)
"""KoLeo loss kernel for Trainium2 (8 NeuronCores, data-parallel rows).

reference semantics:
    x = l2_normalize(student_output)            # [B, D]
    dots = x @ x.T ; dots[i, i] = -1
    I = argmax(dots, 1)
    loss = -mean(log(||x - x[I] + eps|| + eps))

Since rows are unit-norm, ||x_i - x_j|| = sqrt(2 - 2 * dot(x_i, x_j)), so the
nearest-neighbor distance depends only on the max off-diagonal dot:
    loss = -0.5 * mean(ln(2 - 2 * max_j!=i dots[i, j]))
(the eps terms contribute ~1e-8 relative and are dropped).

Sharding: each core gets the full x^T, column-rotated so its own 1024 rows
come first, computes its [1024, 8192] slice of the gram matrix in bf16, and
reduces to a scalar partial sum of ln(2 - 2*maxdot). The rotation makes the
diagonal location core-invariant, so one SPMD program serves all 8 cores.
Host sums the 8 partials.

Per-core device schedule:
  1. cast-DMA x^T f32 -> bf16 SBUF (4 tiles of [128, 8192])
  2. xsq = x*x (ACT), column norms via ones-matmul (PE, broadcasts the sums
     across partitions), inv = exp(-0.5*ln(norm2)) (ACT; Rsqrt is banned)
  3. normalize x in place: x *= inv (DVE)
  4. gram slice: 8 row-tiles x 4 col-groups of [128, 2048] PSUM, K=4 matmuls
     per 512-slice; diagonal killed by one extra I.T @ (-64 shifted I) matmul
  5. row max per col-group (DVE reduce from PSUM), ln(2-2*max) (ACT),
     sum across rows (DVE + gpsimd partition reduce), scalar partial out
"""

import numpy as np
import ml_dtypes

import concourse.bacc as bacc
import concourse.tile as tile
from concourse import mybir, bass_isa
from concourse.bass_utils import run_bass_kernel_spmd

B, D = 8192, 512
N_CORES = 8
ROWS = B // N_CORES          # 1024 rows per core
P = 128                      # SBUF partitions
KT = D // P                  # 4 contraction tiles
M_TILES = ROWS // P          # 8 output row tiles
NT = 512                     # matmul moving free dim
CG = 2048                    # column-group width for the load/norm pipeline
N_CGROUPS = B // CG          # 4
GW = 1024                    # gram PSUM tile width (2 banks)
NG = B // GW                 # 8 gram column groups
DIAG_C = 64.0                # diagonal kill constant

F32 = mybir.dt.float32
BF16 = mybir.dt.bfloat16
AF = mybir.ActivationFunctionType
ALU = mybir.AluOpType

_CACHE: dict = {}


def _build():
    nc = bacc.Bacc(
        "TRN2", target_bir_lowering=False, debug=False, num_devices=N_CORES
    )
    xt = nc.declare_dram_parameter("xt", [D, B], F32, isOutput=False)
    ident = nc.declare_dram_parameter("ident", [P, P], BF16, isOutput=False)
    # ebig[p, 384 + p] = -DIAG_C, zero elsewhere; slicing [384-off : 896-off]
    # yields a [P, NT] tile with -DIAG_C at [p, off + p]
    ebig = nc.declare_dram_parameter("ebig", [P, NT + 3 * P], BF16, isOutput=False)
    partial = nc.declare_dram_parameter("partial", [1, 1], F32, isOutput=True)

    with tile.TileContext(nc) as tc:
        with (
            tc.tile_pool(name="big", bufs=1) as big,
            tc.tile_pool(name="work", bufs=2) as work,
            tc.tile_pool(name="small", bufs=2) as small,
        ):
            ident_sb = big.tile([P, P], BF16, name="ident_sb", tag="ident_sb")
            ebig_sb = big.tile([P, NT + 3 * P], BF16, name="ebig_sb", tag="ebig_sb")
            ones_sb = big.tile([P, P], BF16, name="ones_sb", tag="ones_sb")
            nc.sync.dma_start(ident_sb[:], ident[:])
            nc.sync.dma_start(ebig_sb[:], ebig[:])
            nc.gpsimd.memset(ones_sb[:], 1.0)
            two_sb = small.tile([P, 1], F32, name="two_sb", tag="two_sb")
            nc.gpsimd.memset(two_sb[:], 2.0)

            # --- load x^T (f32 DRAM -> bf16 SBUF cast during DMA), in
            # column-group chunks so later stages can pipeline by cg ---
            xbf = [
                big.tile([P, B], BF16, name=f"xbf{k}", tag=f"xbf{k}")
                for k in range(KT)
            ]
            for cg in range(N_CGROUPS):
                cs = slice(cg * CG, (cg + 1) * CG)
                for k in range(KT):
                    nc.gpsimd.dma_start(
                        xbf[k][:, cs], xt[k * P : (k + 1) * P, cs]
                    )

            xsq = [
                big.tile([P, B], BF16, name=f"xsq{k}", tag=f"xsq{k}")
                for k in range(KT)
            ]
            inv = big.tile([P, B], BF16, name="inv", tag="inv")
            loglist = small.tile([P, M_TILES], F32, name="loglist", tag="loglist")
            # per (mi, 1024-wide column group) partial row-maxes
            maxall = small.tile([P, M_TILES * NG], F32, name="maxall", tag="maxall")

            with (
                tc.tile_pool(name="npsum", bufs=2, space="PSUM") as npsum,
                tc.tile_pool(name="gpsum", bufs=3, space="PSUM") as gpsum,
            ):
                for cg in range(N_CGROUPS):
                    cs = slice(cg * CG, (cg + 1) * CG)
                    # squared entries (DVE: keeping ACT to Ln/Exp only avoids
                    # activation-table thrash between sqrt_* and ln/exp sets)
                    for k in range(KT):
                        nc.vector.tensor_mul(xsq[k][:, cs], xbf[k][:, cs], xbf[k][:, cs])
                    # column norms broadcast across partitions via ones-matmul:
                    # norm2[p, j] = sum_d x[d, j]^2; inv = exp(-0.5*ln(norm2))
                    for c in range(CG // NT):
                        col0 = cg * CG + c * NT
                        nps = npsum.tile([P, NT], F32, name="nps", tag="nps")
                        for k in range(KT):
                            nc.tensor.matmul(
                                nps[:],
                                ones_sb[:],
                                xsq[k][:, col0 : col0 + NT],
                                start=(k == 0),
                                stop=(k == KT - 1),
                            )
                        lntmp = work.tile([P, NT], F32, name="lntmp", tag="lntmp")
                        nc.scalar.activation(lntmp[:], nps[:], AF.Ln)
                        nc.scalar.activation(
                            inv[:, col0 : col0 + NT], lntmp[:], AF.Exp, scale=-0.5
                        )
                    # normalize in place: x[d, j] *= inv[j] (inv row-constant).
                    # cg0 gates the first matmuls (lhsT lives in cg0 columns),
                    # so it runs on the faster DVE; later cgs overlap with
                    # matmuls of the previous cg and go to the idle GpSimd.
                    mul_eng = nc.vector if cg == 0 else nc.gpsimd
                    for k in range(KT):
                        mul_eng.tensor_mul(
                            xbf[k][:, cs], xbf[k][:, cs], inv[:, cs]
                        )
                    # gram slice rows x this column group, then row-max.
                    # G tiles are [128, 1024] (2 PSUM banks): 2 halves per cg.
                    for h in range(2):
                        for mi in range(M_TILES):
                            g = gpsum.tile([P, GW], F32, name="g", tag="g")
                            base = cg * (CG // NT) + h * (GW // NT)
                            diag_c2 = (mi * P) // NT - base  # -1ish if not here
                            # k outer: one LDWEIGHTS serves both 512-slices
                            for k in range(KT):
                                for c2 in range(GW // NT):
                                    nc.tensor.matmul(
                                        g[:, c2 * NT : (c2 + 1) * NT],
                                        xbf[k][:, mi * P : (mi + 1) * P],
                                        xbf[k][:, (base + c2) * NT : (base + c2 + 1) * NT],
                                        start=(k == 0),
                                        stop=(k == KT - 1 and c2 != diag_c2),
                                    )
                            if 0 <= diag_c2 < GW // NT:
                                off = (mi * P) % NT
                                # adds -64 at diag position [p, off+p]
                                nc.tensor.matmul(
                                    g[:, diag_c2 * NT : (diag_c2 + 1) * NT],
                                    ident_sb[:],
                                    ebig_sb[:, 3 * P - off : 3 * P - off + NT],
                                    start=False,
                                    stop=True,
                                )
                            ng = cg * 2 + h  # 1024-wide group index, 0..7
                            nc.vector.reduce_max(
                                maxall[:, mi * NG + ng : mi * NG + ng + 1],
                                g[:],
                                axis=mybir.AxisListType.X,
                            )

                for mi in range(M_TILES):
                    rowmax = small.tile([P, 1], F32, name="rowmax", tag="rowmax")
                    nc.vector.reduce_max(
                        rowmax[:],
                        maxall[:, mi * NG : (mi + 1) * NG],
                        axis=mybir.AxisListType.X,
                    )
                    # ln(2 - 2*maxdot) = 2*ln(nearest-neighbor distance)
                    nc.scalar.activation(
                        loglist[:, mi : mi + 1],
                        rowmax[:],
                        AF.Ln,
                        bias=two_sb[:],
                        scale=-2.0,
                    )

            # --- final reduction to one scalar per core ---
            sumlog = small.tile([P, 1], F32, name="sumlog", tag="sumlog")
            nc.vector.reduce_sum(
                sumlog[:], loglist[:], axis=mybir.AxisListType.X
            )
            total = small.tile([P, 1], F32, name="total", tag="total")
            nc.gpsimd.partition_all_reduce(
                total[:], sumlog[:], P, bass_isa.ReduceOp.add
            )
            nc.sync.dma_start(partial[:], total[0:1, 0:1])

    nc.finalize()
    return nc


def _get_nc():
    if "nc" not in _CACHE:
        _CACHE["nc"] = _build()
    return _CACHE["nc"]


def _in_maps(x: np.ndarray) -> list[dict]:
    ident = np.eye(P, dtype=np.float32).astype(ml_dtypes.bfloat16)
    ebig = np.zeros((P, NT + 3 * P), dtype=np.float32)
    ebig[np.arange(P), 3 * P + np.arange(P)] = -DIAG_C
    ebig = ebig.astype(ml_dtypes.bfloat16)
    maps = []
    for m in range(N_CORES):
        xrot = np.concatenate([x[m * ROWS :], x[: m * ROWS]], axis=0)
        maps.append(
            {
                "xt": np.ascontiguousarray(xrot.T),
                "ident": ident,
                "ebig": ebig,
            }
        )
    return maps


def run_kernel(x: np.ndarray, **spmd_kwargs):
    """Returns (loss_scalar_f32, BassKernelResults)."""
    res = run_bass_kernel_spmd(
        _get_nc(), _in_maps(x), core_ids=list(range(N_CORES)), **spmd_kwargs
    )
    s = sum(float(res.results[m]["partial"][0, 0]) for m in range(N_CORES))
    loss = np.float32(-0.5 * s / B)
    return np.asarray(loss, dtype=np.float32), res


def kernel(student_output: np.ndarray) -> np.ndarray:
    x = np.ascontiguousarray(np.asarray(student_output, dtype=np.float32))
    loss, _ = run_kernel(x)
    return loss



# revision 6
# speedup vs baseline: 1.1570x; 1.1570x over previous
"""KoLeo loss kernel for Trainium2 (8 NeuronCores, data-parallel rows).

reference semantics:
    x = l2_normalize(student_output)            # [B, D]
    dots = x @ x.T ; dots[i, i] = -1
    I = argmax(dots, 1)
    loss = -mean(log(||x - x[I] + eps|| + eps))

Since rows are unit-norm, ||x_i - x_j|| = sqrt(2 - 2 * dot(x_i, x_j)), so
    loss = -0.5 * mean(ln(2 - 2 * max_j!=i dots[i, j]))
(the eps terms contribute ~1e-8 relative and are dropped).

Sharding: each core gets the full x^T (bf16, host-cast), column-rotated so its
own 1024 rows come first, computes its [1024, 8192] slice of the gram matrix
in fp8-e4m3 with DoubleRow matmuls (2 K-planes per instruction), and reduces
to a scalar partial sum of ln(2 - 2*maxdot). Host sums the 8 partials.

Per-core device schedule (pipelined by 2048-wide column groups):
  1. DMA x^T bf16 -> SBUF (host pre-casts f32->bf16: halves HBM traffic)
  2. xsq8 = x*x (DVE, fp8 out in DoubleRow plane layout)
  3. column norms via fp8 DoubleRow ones-matmul (PE broadcasts sums across
     partitions); inv = exp(-0.5*ln(norm2)) (ACT; single act-table set
     natural_log_exp_and_others is pinned => exactly one ACT_TABLE_LOAD)
  4. normalize: xq = x * inv (DVE, bf16*bf16 -> fp8 DoubleRow planes)
  5. gram slice: per (row-tile, 1024-col group) [128,1024] PSUM, 4 DoubleRow
     matmuls (K=256 each); diagonal killed by one extra bf16 I.T@(-64 shifted)
  6. drain: row-max per PSUM tile, split between a direct DVE reduce_max
     (PSUM f32) and an ACT copy->SBUF bf16 + DVE/GpSimd reduce_max path to
     balance engine load
  7. ln(2-2*max) (ACT), row-sum (DVE), partition-sum via f32 ones-matmul,
     scalar partial out
"""

import numpy as np
import ml_dtypes

import concourse.bacc as bacc
import concourse.hw_specs as hw_specs
import concourse.tile as tile
from concourse import mybir
from concourse.bass_utils import run_bass_kernel_spmd

B, D = 8192, 512
N_CORES = 8
ROWS = B // N_CORES          # 1024 rows per core
P = 128                      # SBUF partitions
KT = D // P                  # 4 contraction k-tiles
KB = KT // 2                 # 2 DoubleRow k-blocks (2 planes each)
M_TILES = ROWS // P          # 8 output row tiles
NT = 512                     # matmul moving free dim (psum bank)
CG = 2048                    # column-group width for the load/norm pipeline
N_CGROUPS = B // CG          # 4
GW = 1024                    # gram PSUM tile width (2 banks)
NG = B // GW                 # 8 gram column groups
DIAG_C = 64.0                # diagonal kill constant
N_WARM = 24                  # PE warm-up matmuls issued under the input DMA

F32 = mybir.dt.float32
BF16 = mybir.dt.bfloat16
FP8 = mybir.dt.float8e4
AF = mybir.ActivationFunctionType
DR = mybir.MatmulPerfMode.DoubleRow

# Drain path per (mi, h) gram tile within a column group: True -> ACT copies
# PSUM->SBUF bf16 and DVE/GpSimd does a cheap bf16 reduce; False -> DVE
# reduces straight from PSUM f32.  ~12 staged / 4 direct balances ACT vs DVE.
STAGED = [True, True, True, False] * 4
# GpSimd tensor_reduce only supports partition-axis reductions, so all
# staged bf16 row-reduces stay on DVE.
GP_REDUCE = [False] * 16

_CACHE: dict = {}


def _pin_act_tables():
    """Restrict the activation-table universe to natural_log_exp_and_others
    (contains ln, exp, copy, square, identity) so the table-load inserter
    emits exactly one ACT_TABLE_LOAD instead of thrashing ln<->exp sets.
    Set positions are preserved so the emitted act_func_set_id still indexes
    act_info.json correctly."""
    orig = hw_specs.get_activation_tables("gen3")
    pinned = {
        name: (fns if name == "natural_log_exp_and_others" else set())
        for name, fns in orig.items()
    }
    bacc.get_activation_tables = lambda arch: pinned


def _build():
    _pin_act_tables()
    nc = bacc.Bacc(
        "TRN2", target_bir_lowering=False, debug=False, num_devices=N_CORES
    )
    xt = nc.declare_dram_parameter("xt", [D, B], BF16, isOutput=False)
    ident = nc.declare_dram_parameter("ident", [P, P], BF16, isOutput=False)
    # ebig[p, 384 + p] = -DIAG_C, zero elsewhere; slicing [384-off : 896-off]
    # yields a [P, NT] tile with -DIAG_C at [p, off + p]
    ebig = nc.declare_dram_parameter("ebig", [P, NT + 3 * P], BF16, isOutput=False)
    ones8 = nc.declare_dram_parameter("ones8", [P, 2, P], FP8, isOutput=False)
    partial = nc.declare_dram_parameter("partial", [1, 1], F32, isOutput=True)

    with tile.TileContext(nc) as tc:
        with (
            tc.tile_pool(name="big", bufs=1) as big,
            tc.tile_pool(name="work", bufs=2) as work,
            tc.tile_pool(name="sq", bufs=4) as sqp,
            tc.tile_pool(name="stage", bufs=4) as stage,
            tc.tile_pool(name="small", bufs=2) as small,
        ):
            ident_sb = big.tile([P, P], BF16, name="ident_sb", tag="ident_sb")
            ebig_sb = big.tile([P, NT + 3 * P], BF16, name="ebig_sb", tag="ebig_sb")
            ones8_sb = big.tile([P, 2, P], FP8, name="ones8_sb", tag="ones8_sb")
            onesf_sb = big.tile([P, 1], F32, name="onesf_sb", tag="onesf_sb")
            nc.sync.dma_start(ident_sb[:], ident[:])
            nc.sync.dma_start(ebig_sb[:], ebig[:])
            nc.sync.dma_start(ones8_sb[:], ones8[:])
            nc.gpsimd.memset(onesf_sb[:], 1.0)
            two_sb = small.tile([P, 1], F32, name="two_sb", tag="two_sb")
            nc.gpsimd.memset(two_sb[:], 2.0)

            xbf = [
                big.tile([P, B], BF16, name=f"xbf{k}", tag=f"xbf{k}")
                for k in range(KT)
            ]
            xq = [
                big.tile([P, 2, B], FP8, name=f"xq{kb}", tag=f"xq{kb}")
                for kb in range(KB)
            ]
            invb = big.tile([P, B], BF16, name="invb", tag="invb")
            loglist = small.tile([P, M_TILES], F32, name="loglist", tag="loglist")
            # per (mi, 1024-wide column group) partial row-maxes
            maxall = small.tile([P, M_TILES * NG], F32, name="maxall", tag="maxall")

            with (
                tc.tile_pool(name="npsum", bufs=1, space="PSUM") as npsum,
                tc.tile_pool(name="gpsum", bufs=3, space="PSUM") as gpsum,
            ):
                # PE warm-up: keep the HAM activity window busy during the
                # initial DMA so gram matmuls run at 2.4 GHz from the start.
                warm = npsum.tile([P, GW], F32, name="warm", tag="nps")
                for _ in range(N_WARM):
                    nc.tensor.matmul(
                        warm[:, 0:P], ident_sb[:], ident_sb[:], start=True, stop=True
                    )

                for cg in range(N_CGROUPS):
                    cs = slice(cg * CG, (cg + 1) * CG)
                    for k in range(KT):
                        nc.gpsimd.dma_start(
                            xbf[k][:, cs], xt[k * P : (k + 1) * P, cs]
                        )
                    # squared entries in DoubleRow plane layout (fp8 out)
                    xsq8 = [
                        sqp.tile([P, 2, CG], FP8, name=f"xsq8_{cg}_{kb}", tag=f"xsq8{kb}")
                        for kb in range(KB)
                    ]
                    for k in range(KT):
                        nc.vector.tensor_mul(
                            xsq8[k // 2][:, k % 2, :], xbf[k][:, cs], xbf[k][:, cs]
                        )
                    # column norms broadcast across partitions via fp8
                    # DoubleRow ones-matmul: norm2[p, j] = sum_d x[d, j]^2
                    for h in range(CG // GW):
                        nps = npsum.tile([P, GW], F32, name="nps", tag="nps")
                        for c in range(GW // NT):
                            c0 = h * GW + c * NT
                            for kb in range(KB):
                                nc.tensor.matmul(
                                    nps[:, c * NT : (c + 1) * NT],
                                    ones8_sb[:],
                                    xsq8[kb][:, :, c0 : c0 + NT],
                                    start=(kb == 0),
                                    stop=(kb == KB - 1),
                                    perf_mode=DR,
                                )
                        # inv = exp(-0.5*ln(norm2)); one pinned table set
                        lntmp = work.tile([P, GW], F32, name="lntmp", tag="lntmp")
                        nc.scalar.activation(lntmp[:], nps[:], AF.Ln)
                        col0 = cg * CG + h * GW
                        nc.scalar.activation(
                            invb[:, col0 : col0 + GW], lntmp[:], AF.Exp, scale=-0.5
                        )
                    # normalize into fp8 DoubleRow planes: xq = x * inv
                    for k in range(KT):
                        nc.vector.tensor_mul(
                            xq[k // 2][:, k % 2, cs], xbf[k][:, cs], invb[:, cs]
                        )
                    # gram slice rows x this column group, then row-max
                    for mi in range(M_TILES):
                        for h in range(CG // GW):
                            g = gpsum.tile([P, GW], F32, name="g", tag="g")
                            # diag block for row-tile mi sits at columns
                            # [mi*128, mi*128+128) -- always cg 0, h 0
                            diag_here = cg == 0 and h == 0
                            diag_c = (mi * P) // NT
                            for kb in range(KB):
                                for c in range(GW // NT):
                                    c0 = cg * CG + h * GW + c * NT
                                    nc.tensor.matmul(
                                        g[:, c * NT : (c + 1) * NT],
                                        xq[kb][:, :, mi * P : (mi + 1) * P],
                                        xq[kb][:, :, c0 : c0 + NT],
                                        start=(kb == 0),
                                        stop=(
                                            kb == KB - 1
                                            and not (diag_here and c == diag_c)
                                        ),
                                        perf_mode=DR,
                                    )
                            if diag_here:
                                off = (mi * P) % NT
                                # adds -DIAG_C at diag position [p, off+p]
                                nc.tensor.matmul(
                                    g[:, diag_c * NT : (diag_c + 1) * NT],
                                    ident_sb[:],
                                    ebig_sb[:, 3 * P - off : 3 * P - off + NT],
                                    start=False,
                                    stop=True,
                                )
                            ng = cg * 2 + h
                            mcol = maxall[:, mi * NG + ng : mi * NG + ng + 1]
                            ti = mi * 2 + h
                            if STAGED[ti]:
                                st = stage.tile([P, GW], BF16, name="st", tag="st")
                                nc.scalar.copy(st[:], g[:])
                                red = nc.gpsimd if GP_REDUCE[ti] else nc.vector
                                red.reduce_max(mcol, st[:], axis=mybir.AxisListType.X)
                            else:
                                nc.vector.reduce_max(
                                    mcol, g[:], axis=mybir.AxisListType.X
                                )

                for mi in range(M_TILES):
                    rowmax = small.tile([P, 1], F32, name="rowmax", tag="rowmax")
                    nc.vector.reduce_max(
                        rowmax[:],
                        maxall[:, mi * NG : (mi + 1) * NG],
                        axis=mybir.AxisListType.X,
                    )
                    # ln(2 - 2*maxdot) = 2*ln(nearest-neighbor distance)
                    nc.scalar.activation(
                        loglist[:, mi : mi + 1],
                        rowmax[:],
                        AF.Ln,
                        bias=two_sb[:],
                        scale=-2.0,
                    )

                # --- final reduction to one scalar per core ---
                sumlog = small.tile([P, 1], F32, name="sumlog", tag="sumlog")
                nc.vector.reduce_sum(
                    sumlog[:], loglist[:], axis=mybir.AxisListType.X
                )
                # partition sum via f32 matmul: [1,1] = sumlog.T @ ones
                tot = npsum.tile([P, GW], F32, name="tot", tag="nps")
                nc.tensor.matmul(
                    tot[0:1, 0:1], sumlog[:], onesf_sb[:], start=True, stop=True
                )
                part_sb = small.tile([1, 1], F32, name="part_sb", tag="part_sb")
                nc.vector.tensor_copy(part_sb[:], tot[0:1, 0:1])
                nc.sync.dma_start(partial[:], part_sb[:])

    nc.finalize()
    return nc


def _get_nc():
    if "nc" not in _CACHE:
        _CACHE["nc"] = _build()
    return _CACHE["nc"]


def _in_maps(x: np.ndarray) -> list[dict]:
    ident = np.eye(P, dtype=np.float32).astype(ml_dtypes.bfloat16)
    ebig = np.zeros((P, NT + 3 * P), dtype=np.float32)
    ebig[np.arange(P), 3 * P + np.arange(P)] = -DIAG_C
    ebig = ebig.astype(ml_dtypes.bfloat16)
    ones8 = np.ones((P, 2, P), dtype=np.float32).astype(ml_dtypes.float8_e4m3)
    xbf = x.astype(ml_dtypes.bfloat16)
    maps = []
    for m in range(N_CORES):
        xrot = np.concatenate([xbf[m * ROWS :], xbf[: m * ROWS]], axis=0)
        maps.append(
            {
                "xt": np.ascontiguousarray(xrot.T),
                "ident": ident,
                "ebig": ebig,
                "ones8": ones8,
            }
        )
    return maps


def run_kernel(x: np.ndarray, **spmd_kwargs):
    """Returns (loss_scalar_f32, BassKernelResults)."""
    res = run_bass_kernel_spmd(
        _get_nc(), _in_maps(x), core_ids=list(range(N_CORES)), **spmd_kwargs
    )
    s = sum(float(res.results[m]["partial"][0, 0]) for m in range(N_CORES))
    loss = np.float32(-0.5 * s / B)
    return np.asarray(loss, dtype=np.float32), res


def kernel(student_output: np.ndarray) -> np.ndarray:
    x = np.ascontiguousarray(np.asarray(student_output, dtype=np.float32))
    loss, _ = run_kernel(x)
    return loss
